# revision 1
# baseline (speedup 1.0000x reference)
"""nn_EquivariantLayer: y = x @ w_table[weight_pattern] + b_table[bias_pattern].

Column-sharded, DVE-built W variant.  Core c computes y[:, 256c:256(c+1)]
for ALL batch rows:

 - W slice [2048, 256] is built on the VECTOR engine by a 64-group
   compare-select sweep over the (host-repacked) pattern:
   term = (pat == g) * t[g] is ONE fused tensor_scalar per group (scalar2
   is a runtime AP into the codebook), then an add pass accumulates.  All
   int16/bf16 operands -> DVE 4x mode; ~160us per slice.  This replaces
   GPSIMD ap_gather, which measures ~30ns per stream slot on this stack
   (~1ms for the slice) and dominated every gather-based design.
 - The pattern is repacked on the host so the accumulator IS the matmul
   rhs layout ([128 part = row-in-k-tile, free = (k, col)]): no fix-up
   DMAs.  Built in 2 chunks of 8 k-tiles so the PE can start on chunk 0
   while chunk 1 builds.
 - No collectives (measured ~57-200us/call here).  x is broadcast: each
   core streams the full bf16 xT (64 MiB) in 2048-row supertiles; bf16
   matmuls (N=256) in k-outer waves of 4 m-tiles (half the PSUM banks per
   wave so evictions overlap).  y is written bf16, upcast on the host.
 - The W buffer is double-buffered across repeat iterations, so in
   steady state the next build overlaps the current matmul.
"""

import numpy as np
import ml_dtypes

import concourse.bass as bass
import concourse.mybir as mybir
import concourse.tile as tile
from concourse import bacc
from concourse.bass_utils import run_bass_kernel_spmd

F32 = mybir.dt.float32
BF16 = mybir.dt.bfloat16
I32 = mybir.dt.int32
I16 = mybir.dt.int16

BATCH, D, NCORES = 16384, 2048, 8
COLS = D // NCORES         # 256 output cols per core
GW, GB = 65, 17
SMB = 2048                 # batch supertile (four 4-bank PSUM waves)
P = 128
NK = D // P                # 16 k-tiles
WCH = 2                    # W build chunks (8 k-tiles each)
CHW = NK * COLS // WCH     # 2048 free-dim width per chunk

_CACHED = {}


def _build_program(repeat=1, mb=BATCH):
    nc = bacc.Bacc("TRN2", target_bir_lowering=False, debug=False,
                   num_devices=NCORES)

    smb = min(SMB, mb)
    ns = mb // smb
    nm = smb // P

    x_in = nc.dram_tensor("x", [D, mb], BF16, kind="ExternalInput").ap()
    pat_in = nc.dram_tensor("pat", [P, NK * COLS], I16, kind="ExternalInput").ap()
    wt_in = nc.dram_tensor("wt", [1, GW], F32, kind="ExternalInput").ap()
    bp_in = nc.dram_tensor("bp", [1, COLS], I32, kind="ExternalInput").ap()
    bt_in = nc.dram_tensor("bt", [1, GB], F32, kind="ExternalInput").ap()
    y_out = nc.dram_tensor("y", [mb, COLS], BF16, kind="ExternalOutput").ap()
    b_dram = nc.dram_tensor("b_dram", [1, COLS], F32).ap()

    with tile.TileContext(nc) as tc:
        with tc.tile_pool(name="const", bufs=1) as cp, \
             tc.tile_pool(name="wbuild", bufs=2) as wb, \
             tc.tile_pool(name="xpool", bufs=2) as xp, \
             tc.tile_pool(name="mm", bufs=4) as mp, \
             tc.tile_pool(name="psum", bufs=8, space="PSUM") as pp:
            # ---- rep-invariant preamble: pattern, codebook, bias ----
            patall = cp.tile([P, NK * COLS], I16)
            nc.sync.dma_start(out=patall[:], in_=pat_in[:])
            wt = cp.tile([P, GW], F32)
            nc.sync.dma_start(out=wt[:], in_=wt_in[:].to_broadcast([P, GW]))

            # bias: b = bt[bp] via 16-partition compare loop
            bp16 = cp.tile([16, COLS // 16], I32)
            nc.sync.dma_start(
                out=bp16[:], in_=bp_in[:].rearrange("a (p f) -> (a p) f", p=16))
            btt = cp.tile([16, GB], F32)
            nc.sync.dma_start(out=btt[:], in_=bt_in[:].to_broadcast([16, GB]))
            acc = cp.tile([16, COLS // 16], F32)
            nc.vector.memset(acc[:], 0.0)
            for gi in range(1, GB):
                mask = cp.tile([16, COLS // 16], F32, name=f"bm{gi}")
                nc.vector.tensor_scalar(
                    out=mask[:], in0=bp16[:], scalar1=float(gi), scalar2=0.0,
                    op0=mybir.AluOpType.is_equal, op1=mybir.AluOpType.add)
                nc.vector.tensor_tensor(
                    out=mask[:], in0=mask[:],
                    in1=btt[:, gi:gi + 1].to_broadcast([16, COLS // 16]),
                    op=mybir.AluOpType.mult)
                nc.vector.tensor_tensor(
                    out=acc[:], in0=acc[:], in1=mask[:],
                    op=mybir.AluOpType.add)
            nc.sync.dma_start(
                out=b_dram[:].rearrange("a (p f) -> (a p) f", p=16), in_=acc[:])
            b1 = cp.tile([1, COLS], F32)
            nc.sync.dma_start(out=b1[:], in_=b_dram[:])
            brow = cp.tile([1, COLS], BF16)
            nc.vector.tensor_copy(out=brow[:], in_=b1[:])
            ones = cp.tile([1, P], BF16)
            nc.vector.memset(ones[:], 1.0)

            for _rep in range(repeat):
                # ---- W build on DVE: wk_all[p, k*256+c] = wt[pat[...]] ----
                wk_all = wb.tile([P, NK * COLS], BF16, tag="wkall")
                for ch in range(WCH):
                    sl = slice(CHW * ch, CHW * (ch + 1))
                    for g in range(1, GW):
                        if g == 1:
                            nc.vector.tensor_scalar(
                                out=wk_all[:, sl], in0=patall[:, sl],
                                scalar1=float(g),
                                scalar2=wt[:, g:g + 1],
                                op0=mybir.AluOpType.is_equal,
                                op1=mybir.AluOpType.mult)
                        else:
                            term = mp.tile([P, CHW], BF16, tag="wterm")
                            nc.vector.tensor_scalar(
                                out=term[:], in0=patall[:, sl],
                                scalar1=float(g),
                                scalar2=wt[:, g:g + 1],
                                op0=mybir.AluOpType.is_equal,
                                op1=mybir.AluOpType.mult)
                            nc.vector.tensor_tensor(
                                out=wk_all[:, sl], in0=wk_all[:, sl],
                                in1=term[:], op=mybir.AluOpType.add)

                # ---- supertiles of 2048 batch rows ----
                for s in range(ns):
                    xts = []
                    for k in range(NK):
                        xt = xp.tile([P, smb], BF16, tag=f"x{k}",
                                     name=f"x{s}_{k}")
                        nc.scalar.dma_start(
                            out=xt[:],
                            in_=x_in[P * k:P * (k + 1), smb * s:smb * (s + 1)])
                        xts.append(xt)

                    def evict(m, ps):
                        ystage = mp.tile([P, COLS], BF16, tag="ystage")
                        nc.scalar.copy(out=ystage[:], in_=ps[:])
                        nc.sync.dma_start(
                            out=y_out[smb * s + P * m:smb * s + P * (m + 1), :],
                            in_=ystage[:])

                    # k-outer waves of 4 m-tiles (half the PSUM banks per
                    # wave, so wave i+1 streams while wave i evictions drain)
                    for w0 in range(0, nm, 4):
                        wn = min(4, nm - w0)
                        ps4 = [pp.tile([P, COLS], F32, tag="ps",
                                       name=f"ps{s}_{m}")
                               for m in range(w0, w0 + wn)]
                        for mi in range(wn):
                            nc.tensor.matmul(ps4[mi][:], ones[:], brow[:],
                                             start=True, stop=False)
                        for k in range(NK):
                            rhs = wk_all[:, COLS * k:COLS * (k + 1)]
                            for mi, m in enumerate(range(w0, w0 + wn)):
                                nc.tensor.matmul(
                                    ps4[mi][:], xts[k][:, P * m:P * (m + 1)],
                                    rhs,
                                    start=False, stop=(k == NK - 1))
                        for mi, m in enumerate(range(w0, w0 + wn)):
                            evict(m, ps4[mi])

    nc.compile()
    return nc


def _get_nc(repeat=1, mb=BATCH):
    key = (repeat, mb)
    if key not in _CACHED:
        _CACHED[key] = _build_program(repeat, mb)
    return _CACHED[key]


def _make_in_maps(x, matrix_params, bias_params, weight_pattern, bias_pattern):
    wt = np.concatenate([np.zeros(1, np.float32),
                         np.asarray(matrix_params, np.float32).reshape(-1)])
    bt = np.concatenate([np.zeros(1, np.float32),
                         np.asarray(bias_params, np.float32).reshape(-1)])
    x = np.asarray(x, np.float32)
    pat = np.asarray(weight_pattern, np.int32)
    bp = np.ascontiguousarray(np.asarray(bias_pattern, np.int32))

    xT = np.ascontiguousarray(x.T.astype(ml_dtypes.bfloat16))   # [D, BATCH]

    in_maps = []
    for dev in range(NCORES):
        ps = pat[:, COLS * dev:COLS * (dev + 1)]     # [D, 256]
        patall = np.ascontiguousarray(
            ps.reshape(NK, P, COLS).transpose(1, 0, 2).reshape(P, NK * COLS)
        ).astype(np.int16)
        in_maps.append({
            "x": xT,
            "pat": patall,
            "wt": wt.reshape(1, GW),
            "bp": bp[COLS * dev:COLS * (dev + 1)].reshape(1, COLS),
            "bt": bt.reshape(1, GB),
        })
    return in_maps


def kernel(x, matrix_params, bias_params, weight_pattern, bias_pattern):
    nc = _get_nc()
    in_maps = _make_in_maps(x, matrix_params, bias_params,
                            weight_pattern, bias_pattern)
    res = run_bass_kernel_spmd(nc, in_maps, list(range(NCORES)))
    return np.concatenate(
        [res.results[c]["y"].astype(np.float32) for c in range(NCORES)], axis=1)



# revision 3
# speedup vs baseline: 1.4344x; 1.4344x over previous
"""nn_EquivariantLayer: y = x @ w_table[weight_pattern] + b_table[bias_pattern].

Data-parallel bf16 design (core c computes y[2048c:2048(c+1), :]):

 - W is expanded from the codebook on the HOST (a numpy gather) and shipped
   bf16; the kernel is then a pure GEMM.  The 16 MiB replicated W slab is
   DMA'd once in the preamble (like the baseline's pattern tables); per
   iteration each core streams only its 8.4 MiB x slice and writes 8.4 MiB
   of y -- far under the PE time, so the kernel is tensor-bound.
 - Matmuls use the full 512-wide moving dim (vs 256 in the column-sharded
   baseline), halving per-instruction overhead.  Stationary x tiles are
   reused across the 4 n-blocks (s-outer, c-inner order) so consecutive
   instructions share their LD_WEIGHTS.
 - Bias is added at PSUM eviction on the DVE (tensor_tensor add with an
   f32 bias slab), so chains are pure k-accumulations; no ones-matmul.
 - x arrives in 2 half-slabs (double-buffered) so the next iteration's
   first half overlaps the current iteration's second-half compute.
"""

import numpy as np
import ml_dtypes

import concourse.bass as bass
import concourse.mybir as mybir
import concourse.tile as tile
from concourse import bacc
from concourse.bass_utils import run_bass_kernel_spmd

F32 = mybir.dt.float32
BF16 = mybir.dt.bfloat16

BATCH, D, NCORES = 16384, 2048, 8
MB = BATCH // NCORES       # 2048 rows per core
P = 128
NK = D // P                # 16 k-subtiles
NM = MB // P               # 16 m-tiles
NB = D // 512              # 4 n-blocks of 512
HM = MB // 2               # half-slab rows (1024)

_CACHED = {}


def _build_program(repeat=1):
    nc = bacc.Bacc("TRN2", target_bir_lowering=False, debug=False,
                   num_devices=NCORES)

    xs_in = nc.dram_tensor("xs", [P, NK * MB], BF16, kind="ExternalInput").ap()
    ws_in = nc.dram_tensor("ws", [P, NK * D], BF16, kind="ExternalInput").ap()
    br_in = nc.dram_tensor("br", [1, D], F32, kind="ExternalInput").ap()
    y_out = nc.dram_tensor("y", [MB, D], BF16, kind="ExternalOutput").ap()

    with tile.TileContext(nc) as tc:
        with tc.tile_pool(name="const", bufs=1) as cp, \
             tc.tile_pool(name="xpool", bufs=2) as xp, \
             tc.tile_pool(name="stage", bufs=4) as sp, \
             tc.tile_pool(name="psum", bufs=8, space="PSUM") as pp:
            # ---- preamble (once): W slab, bias slab ----
            wt = cp.tile([P, NK, D], BF16)
            nc.sync.dma_start(out=wt[:], in_=ws_in[:].rearrange(
                "p (s n) -> p s n", s=NK))
            bt = cp.tile([P, D], F32)
            nc.sync.dma_start(out=bt[:], in_=br_in[:].to_broadcast([P, D]))

            for _rep in range(repeat):
                for h in range(2):
                    xt = xp.tile([P, NK, HM], BF16, tag="xt", name=f"x{h}")
                    nc.scalar.dma_start(
                        out=xt[:],
                        in_=xs_in[:].rearrange("p (s m) -> p s m", s=NK)
                        [:, :, HM * h:HM * (h + 1)])
                    for mh in range(NM // 2):
                        m = h * (NM // 2) + mh
                        pss = [pp.tile([P, 512], F32, tag="ps",
                                       name=f"ps{m}_{c}") for c in range(NB)]
                        for s in range(NK):
                            lhs = xt[:, s:s + 1, P * mh:P * (mh + 1)]
                            for c in range(NB):
                                nc.tensor.matmul(
                                    pss[c][:], lhs,
                                    wt[:, s:s + 1, 512 * c:512 * (c + 1)],
                                    start=(s == 0), stop=(s == NK - 1))
                        for c in range(NB):
                            st = sp.tile([P, 512], BF16, tag="st",
                                         name=f"st{m}_{c}")
                            nc.vector.tensor_tensor(
                                out=st[:], in0=pss[c][:],
                                in1=bt[:, 512 * c:512 * (c + 1)],
                                op=mybir.AluOpType.add)
                            nc.sync.dma_start(
                                out=y_out[P * m:P * (m + 1),
                                          512 * c:512 * (c + 1)],
                                in_=st[:])

    nc.compile()
    return nc


def _get_nc(repeat=1):
    key = repeat
    if key not in _CACHED:
        _CACHED[key] = _build_program(repeat)
    return _CACHED[key]


def _make_in_maps(x, matrix_params, bias_params, weight_pattern, bias_pattern):
    bf16 = ml_dtypes.bfloat16
    t = np.concatenate([np.zeros(1, np.float32),
                        np.asarray(matrix_params, np.float32).reshape(-1)])
    btab = np.concatenate([np.zeros(1, np.float32),
                           np.asarray(bias_params, np.float32).reshape(-1)])
    pat = np.asarray(weight_pattern, np.int32)
    brow = btab[np.asarray(bias_pattern, np.int32)].reshape(1, D)
    brow = np.ascontiguousarray(brow.astype(np.float32))

    W = t.astype(bf16)[pat]                       # [D, D] bf16 host gather
    ws = np.ascontiguousarray(
        W.reshape(NK, P, D).transpose(1, 0, 2)).reshape(P, NK * D)

    x = np.asarray(x, np.float32).astype(bf16)    # [BATCH, D] bf16
    in_maps = []
    for dev in range(NCORES):
        xc = x[MB * dev:MB * (dev + 1)]           # [MB, D]
        xsl = np.ascontiguousarray(
            xc.T.reshape(NK, P, MB).transpose(1, 0, 2)).reshape(P, NK * MB)
        in_maps.append({"xs": xsl, "ws": ws, "br": brow})
    return in_maps


def kernel(x, matrix_params, bias_params, weight_pattern, bias_pattern):
    nc = _get_nc()
    in_maps = _make_in_maps(x, matrix_params, bias_params,
                            weight_pattern, bias_pattern)
    res = run_bass_kernel_spmd(nc, in_maps, list(range(NCORES)))
    return np.concatenate(
        [res.results[c]["y"].astype(np.float32) for c in range(NCORES)], axis=0)
